# revision 3
# baseline (speedup 1.0000x reference)
"""Trainium2 Bass kernel for nn_CriticGCN (2-layer GCN critic, 50000 nodes,
800000 edges, 8 NeuronCores).

Algebraic reformulation (exact):
  out1 = A @ X W1 + b1 with A = GCN-normalized adjacency (incl. self loops)
  z    = relu(out1) @ (W2 @ W3)          (layer-2 feature dim collapsed to 1)
  y    = A @ z + (b2 @ W3 + b3)
  A @ v per node d: dis[d] * (sum_{e: s->d} dis[s]*v[s] + dis[d]*v[d])

Layout strategy (no one-hot matmuls, no segment matmuls):
  - Nodes are relabeled by (lo_indeg, hi_indeg) lexicographic sort so each
    window of 128 dst nodes has near-uniform in-degree. Consecutive sorted
    windows are dealt round-robin to the 8 cores (load balance + identical
    shapes for SPMD).
  - Each dst node gets a fixed number of edge slots (window max degree incl.
    self loop); slot (p, t) of a window holds a gathered 256B row of the
    pre-scaled source table U = X*dis (or zdis = z*dis in pass 2). Padding
    slots point at an all-zeros table row.
  - The segment sum is then a plain free-dim tensor_reduce on the Vector
    engine: seg[p, f] = sum_t gathered[p, t, f]. Self loops are extra slots.
  - Gathers are grouped 7 windows per dma_gather call to amortize the SWDGE
    fixed cost; lo/hi table split works around the int16 index range.
  - Dense chain per 4 windows: transpose seg to [20, 512] via TensorE, then
    x1 = relu(W1^T g + b1) and z = w23^T x1 accumulated in PSUM.
"""
import numpy as np
import concourse.bacc as bacc
import concourse.mybir as mybir
import concourse.tile as tile
from concourse.bass_utils import run_bass_kernel_spmd

P = 128
NCORES = 8
WPC = 49                 # windows per core
NWIN = NCORES * WPC      # 392
NPAD = NWIN * P          # 50176
N = 50000
LOMAX = 32765            # node ids <= LOMAX live in the lo table region
ZLO = 32766              # lo region all-zeros pad row
HIBASE = 32767           # table row offset of the hi region
NHI = N - (LOMAX + 1)    # 17234 hi nodes (ids 32766..49999)
ZHI = NHI                # hi region pad idx (row HIBASE+NHI is zeros)
NROWS = HIBASE + NHI + 1 # 50002
D = 64                   # table row: 64 f32 = 256 B (dma_gather minimum)
DF = 20
GDW = 7                  # windows per dma_gather call
NGG = WPC // GDW         # 7 gather groups
GRP = 4                  # windows per dense-chain group
NGRP = (WPC + GRP - 1) // GRP
ZCOLS = NGRP * GRP * P

IDENT = np.eye(P, dtype=np.float32)


def _wrap16(a):
    return np.tile(a.astype(np.int16).reshape(-1, 16).T, (8, 1))


def _preprocess(state, edge_attr, edge_index):
    X = np.concatenate([state.reshape(-1, edge_attr.shape[1]),
                        edge_attr], 0).astype(np.float32)
    src = edge_index[0].astype(np.int64)
    dst = edge_index[1].astype(np.int64)

    deg = np.bincount(dst, minlength=N)
    dis = (1.0 / np.sqrt(deg + 1.0)).astype(np.float32)

    # edge lists incl. self loops, split by src table region, grouped by dst
    esrc = np.concatenate([src, np.arange(N, dtype=np.int64)])
    edst = np.concatenate([dst, np.arange(N, dtype=np.int64)])
    is_hi = esrc > LOMAX
    lo_src, lo_dst = esrc[~is_hi], edst[~is_hi]
    hi_src, hi_dst = esrc[is_hi], edst[is_hi]
    o = np.argsort(lo_dst, kind="stable")
    lo_src = lo_src[o]
    o = np.argsort(hi_dst, kind="stable")
    hi_src = hi_src[o] - (LOMAX + 1)
    lo_cnt = np.bincount(lo_dst, minlength=N)
    hi_cnt = np.bincount(hi_dst, minlength=N)
    lo_off = np.concatenate([[0], np.cumsum(lo_cnt)])
    hi_off = np.concatenate([[0], np.cumsum(hi_cnt)])

    # degree-homogeneous windows: sort by (lo_cnt, hi_cnt), pads first
    order = np.lexsort((hi_cnt, lo_cnt))
    member = np.full(NPAD, -1, dtype=np.int64)
    member[NPAD - N:] = order

    # sorted window j -> core j%8, within-core index j//8
    # shapes must match across cores (SPMD): KLO/KHI = max over the 8 cores
    mem2 = member.reshape(NWIN, P)
    klo_w = np.zeros(NWIN, dtype=np.int64)
    khi_w = np.zeros(NWIN, dtype=np.int64)
    for j in range(NWIN):
        m = mem2[j]
        real = m >= 0
        if real.any():
            klo_w[j] = lo_cnt[m[real]].max()
            khi_w[j] = hi_cnt[m[real]].max()
    KLO = np.maximum(klo_w.reshape(WPC, NCORES).max(1), 1)  # [49] per wi
    KHI = np.maximum(khi_w.reshape(WPC, NCORES).max(1), 1)

    def fill_rect(m, cnt_tab, off_tab, src_tab, k, padval):
        arr = np.full((k, P), padval, dtype=np.int64)
        if k == 0:
            return arr
        real = np.where(m >= 0)[0]
        mr = m[real]
        cnts = cnt_tab[mr]
        tot = int(cnts.sum())
        if tot == 0:
            return arr
        base = np.repeat(off_tab[mr], cnts)
        csum = np.concatenate([[0], np.cumsum(cnts)])[:-1]
        within = np.arange(tot) - np.repeat(csum, cnts)
        arr[within, np.repeat(real, cnts)] = src_tab[base + within]
        return arr

    per_core = []
    disw_all = np.where(member >= 0, dis[np.clip(member, 0, None)],
                        1.0).astype(np.float32).reshape(NWIN, P)
    for k in range(NCORES):
        ilo_parts, ihi_parts = [], []
        for wi in range(WPC):
            j = wi * NCORES + k
            m = mem2[j]
            ilo_parts.append(
                fill_rect(m, lo_cnt, lo_off, lo_src, KLO[wi], ZLO).reshape(-1))
            ihi_parts.append(
                fill_rect(m, hi_cnt, hi_off, hi_src, KHI[wi], ZHI).reshape(-1))
        # one wrapped idx table per gather group, concatenated along cols
        ilo_cols, ihi_cols = [], []
        for g in range(NGG):
            lo_flat = np.concatenate(ilo_parts[g * GDW:(g + 1) * GDW])
            hi_flat = np.concatenate(ihi_parts[g * GDW:(g + 1) * GDW])
            ilo_cols.append(_wrap16(lo_flat))
            ihi_cols.append(_wrap16(hi_flat))
        disw = disw_all[np.arange(WPC) * NCORES + k].T.copy()  # [128, 49]
        per_core.append(dict(ilo=np.concatenate(ilo_cols, axis=1),
                             ihi=np.concatenate(ihi_cols, axis=1),
                             disw=disw))

    # pre-scaled gather table U = X*dis in table-row layout
    U = X * dis[:, None]
    Upad = np.zeros((NROWS, D), dtype=np.float32)
    ids = np.arange(N)
    rows = np.where(ids <= LOMAX, ids, ids + 1)
    Upad[rows, :DF] = U

    return dict(per_core=per_core, member=member, dis=dis, rows=rows,
                KLO=KLO, KHI=KHI)


def _build_common(nc, KLO, KHI):
    """Declare the shared dram tensors + idx/disw constants; returns dict."""
    f32 = mybir.dt.float32
    nlo16 = int(sum(KLO)) * P // 16
    nhi16 = int(sum(KHI)) * P // 16
    t = {}
    t["xpad"] = nc.dram_tensor("xpad", [NROWS, D], f32, kind="ExternalInput")
    t["ilo"] = nc.dram_tensor("ilo", [128, nlo16], mybir.dt.int16,
                              kind="ExternalInput")
    t["ihi"] = nc.dram_tensor("ihi", [128, nhi16], mybir.dt.int16,
                              kind="ExternalInput")
    t["disw"] = nc.dram_tensor("disw", [P, WPC], f32, kind="ExternalInput")
    return t


def build_pass1(KLO, KHI, reps=1):
    KLO = [int(v) for v in KLO]
    KHI = [int(v) for v in KHI]
    KLOG = [sum(KLO[g * GDW:(g + 1) * GDW]) for g in range(NGG)]
    KHIG = [sum(KHI[g * GDW:(g + 1) * GDW]) for g in range(NGG)]
    KLOGM, KHIGM = max(KLOG), max(KHIG)
    lo_go = np.concatenate([[0], np.cumsum(KLOG)]) * P // 16
    hi_go = np.concatenate([[0], np.cumsum(KHIG)]) * P // 16

    nc = bacc.Bacc("TRN2", target_bir_lowering=False, debug=False,
                   num_devices=NCORES)
    f32 = mybir.dt.float32
    t = _build_common(nc, KLO, KHI)
    ident_d = nc.dram_tensor("ident", [P, P], f32, kind="ExternalInput")
    w1_d = nc.dram_tensor("w1", [DF, 500], f32, kind="ExternalInput")
    b1_d = nc.dram_tensor("b1", [125, 4], f32, kind="ExternalInput")
    w23_d = nc.dram_tensor("w23", [125, 4], f32, kind="ExternalInput")
    z_d = nc.dram_tensor("z", [1, ZCOLS], f32, kind="ExternalOutput")

    Relu = mybir.ActivationFunctionType.Relu
    with tile.TileContext(nc) as tc:
        with tc.tile_pool(name="cst", bufs=1) as cst, \
             tc.tile_pool(name="g", bufs=2) as gpl, \
             tc.tile_pool(name="wk", bufs=3) as wk, \
             tc.tile_pool(name="ps", bufs=2, space="PSUM") as ps:
            ilo_t = cst.tile([128, t["ilo"].shape[1]], mybir.dt.int16)
            nc.sync.dma_start(out=ilo_t[:], in_=t["ilo"][:])
            ihi_t = cst.tile([128, t["ihi"].shape[1]], mybir.dt.int16)
            nc.sync.dma_start(out=ihi_t[:], in_=t["ihi"][:])
            disw_t = cst.tile([P, WPC], f32)
            nc.sync.dma_start(out=disw_t[:], in_=t["disw"][:])
            ident_t = cst.tile([P, P], f32)
            nc.sync.dma_start(out=ident_t[:], in_=ident_d[:])
            w1_t = cst.tile([DF, 500], f32)
            nc.sync.dma_start(out=w1_t[:], in_=w1_d[:])
            b1_t = cst.tile([125, 4], f32)
            nc.sync.dma_start(out=b1_t[:], in_=b1_d[:])
            w23_t = cst.tile([125, 4], f32)
            nc.sync.dma_start(out=w23_t[:], in_=w23_d[:])
            z_sb = cst.tile([1, ZCOLS], f32)

            for _rep in range(reps):
                gt_ps = None
                for g in range(NGG):
                    glo = gpl.tile([P, KLOGM, D], f32, tag="glo")
                    nc.gpsimd.dma_gather(
                        out_ap=glo[:, :KLOG[g], :],
                        in_ap=t["xpad"][:HIBASE, :],
                        idxs_ap=ilo_t[:, lo_go[g]:lo_go[g + 1]],
                        num_idxs=KLOG[g] * P, num_idxs_reg=KLOG[g] * P,
                        elem_size=D, single_packet=False)
                    ghi = gpl.tile([P, KHIGM, D], f32, tag="ghi")
                    nc.gpsimd.dma_gather(
                        out_ap=ghi[:, :KHIG[g], :],
                        in_ap=t["xpad"][HIBASE:, :],
                        idxs_ap=ihi_t[:, hi_go[g]:hi_go[g + 1]],
                        num_idxs=KHIG[g] * P, num_idxs_reg=KHIG[g] * P,
                        elem_size=D, single_packet=False)
                    olo, ohi = 0, 0
                    for wl in range(GDW):
                        w = g * GDW + wl
                        if w % GRP == 0:
                            gt_ps = ps.tile([DF, GRP * P], f32, space="PSUM",
                                            tag="gtps")
                        klo, khi = KLO[w], KHI[w]
                        seg = wk.tile([P, DF], f32, tag="seg")
                        rhi = wk.tile([P, DF], f32, tag="rhi")
                        nc.vector.tensor_reduce(
                            out=seg[:],
                            in_=glo[:, olo:olo + klo, :DF]
                                .rearrange("p k f -> p f k"),
                            axis=mybir.AxisListType.X, op=mybir.AluOpType.add)
                        nc.vector.tensor_reduce(
                            out=rhi[:],
                            in_=ghi[:, ohi:ohi + khi, :DF]
                                .rearrange("p k f -> p f k"),
                            axis=mybir.AxisListType.X, op=mybir.AluOpType.add)
                        olo += klo
                        ohi += khi
                        nc.vector.tensor_tensor(out=seg[:], in0=seg[:],
                                                in1=rhi[:],
                                                op=mybir.AluOpType.add)
                        nc.vector.tensor_tensor(
                            out=seg[:], in0=seg[:],
                            in1=disw_t[:, w:w + 1].to_broadcast([P, DF]),
                            op=mybir.AluOpType.mult)
                        wi = w % GRP
                        nc.tensor.transpose(out=gt_ps[:, wi * P:(wi + 1) * P],
                                            in_=seg[:], identity=ident_t[:])
                        if w % GRP == GRP - 1 or w == WPC - 1:
                            grp = w // GRP
                            gt_sb = wk.tile([DF, GRP * P], f32, tag="gtsb")
                            nc.vector.tensor_copy(out=gt_sb[:], in_=gt_ps[:])
                            zrow = ps.tile([1, GRP * P], f32, space="PSUM",
                                           tag="zrow")
                            for c in range(4):
                                o1 = ps.tile([125, GRP * P], f32,
                                             space="PSUM", tag="o1")
                                nc.tensor.matmul(
                                    out=o1[:],
                                    lhsT=w1_t[:, c * 125:(c + 1) * 125],
                                    rhs=gt_sb[:], start=True, stop=True)
                                x1 = wk.tile([125, GRP * P], f32, tag="x1")
                                nc.scalar.activation(x1[:], o1[:], Relu,
                                                     bias=b1_t[:, c:c + 1],
                                                     scale=1.0)
                                nc.tensor.matmul(out=zrow[:],
                                                 lhsT=w23_t[:, c:c + 1],
                                                 rhs=x1[:], start=(c == 0),
                                                 stop=(c == 3))
                            nc.vector.tensor_copy(
                                out=z_sb[:, grp * GRP * P:(grp + 1) * GRP * P],
                                in_=zrow[:])
            nc.sync.dma_start(out=z_d[:], in_=z_sb[:])
    nc.compile()
    return nc


def build_pass2(KLO, KHI, reps=1):
    KLO = [int(v) for v in KLO]
    KHI = [int(v) for v in KHI]
    KLOG = [sum(KLO[g * GDW:(g + 1) * GDW]) for g in range(NGG)]
    KHIG = [sum(KHI[g * GDW:(g + 1) * GDW]) for g in range(NGG)]
    KLOGM, KHIGM = max(KLOG), max(KHIG)
    lo_go = np.concatenate([[0], np.cumsum(KLOG)]) * P // 16
    hi_go = np.concatenate([[0], np.cumsum(KHIG)]) * P // 16

    nc = bacc.Bacc("TRN2", target_bir_lowering=False, debug=False,
                   num_devices=NCORES)
    f32 = mybir.dt.float32
    t = _build_common(nc, KLO, KHI)
    y_d = nc.dram_tensor("y", [P, WPC], f32, kind="ExternalOutput")

    with tile.TileContext(nc) as tc:
        with tc.tile_pool(name="cst", bufs=1) as cst, \
             tc.tile_pool(name="g", bufs=2) as gpl, \
             tc.tile_pool(name="wk", bufs=3) as wk:
            ilo_t = cst.tile([128, t["ilo"].shape[1]], mybir.dt.int16)
            nc.sync.dma_start(out=ilo_t[:], in_=t["ilo"][:])
            ihi_t = cst.tile([128, t["ihi"].shape[1]], mybir.dt.int16)
            nc.sync.dma_start(out=ihi_t[:], in_=t["ihi"][:])
            disw_t = cst.tile([P, WPC], f32)
            nc.sync.dma_start(out=disw_t[:], in_=t["disw"][:])
            y_sb = cst.tile([P, WPC], f32)

            for _rep in range(reps):
                for g in range(NGG):
                    glo = gpl.tile([P, KLOGM, D], f32, tag="glo")
                    nc.gpsimd.dma_gather(
                        out_ap=glo[:, :KLOG[g], :],
                        in_ap=t["xpad"][:HIBASE, :],
                        idxs_ap=ilo_t[:, lo_go[g]:lo_go[g + 1]],
                        num_idxs=KLOG[g] * P, num_idxs_reg=KLOG[g] * P,
                        elem_size=D, single_packet=False)
                    ghi = gpl.tile([P, KHIGM, D], f32, tag="ghi")
                    nc.gpsimd.dma_gather(
                        out_ap=ghi[:, :KHIG[g], :],
                        in_ap=t["xpad"][HIBASE:, :],
                        idxs_ap=ihi_t[:, hi_go[g]:hi_go[g + 1]],
                        num_idxs=KHIG[g] * P, num_idxs_reg=KHIG[g] * P,
                        elem_size=D, single_packet=False)
                    olo, ohi = 0, 0
                    for wl in range(GDW):
                        w = g * GDW + wl
                        klo, khi = KLO[w], KHI[w]
                        seg = wk.tile([P, 1], f32, tag="seg")
                        rhi = wk.tile([P, 1], f32, tag="rhi")
                        nc.vector.tensor_reduce(
                            out=seg[:],
                            in_=glo[:, olo:olo + klo, 0:1]
                                .rearrange("p k f -> p f k"),
                            axis=mybir.AxisListType.X, op=mybir.AluOpType.add)
                        nc.vector.tensor_reduce(
                            out=rhi[:],
                            in_=ghi[:, ohi:ohi + khi, 0:1]
                                .rearrange("p k f -> p f k"),
                            axis=mybir.AxisListType.X, op=mybir.AluOpType.add)
                        olo += klo
                        ohi += khi
                        nc.vector.tensor_tensor(out=seg[:], in0=seg[:],
                                                in1=rhi[:],
                                                op=mybir.AluOpType.add)
                        nc.vector.tensor_tensor(
                            out=y_sb[:, w:w + 1], in0=seg[:],
                            in1=disw_t[:, w:w + 1], op=mybir.AluOpType.mult)
            nc.sync.dma_start(out=y_d[:], in_=y_sb[:])
    nc.compile()
    return nc


def pass1_inmaps(pp, Upad, W1, b1, w23):
    maps = []
    for k in range(NCORES):
        c = pp["per_core"][k]
        maps.append({
            "xpad": Upad,
            "ilo": c["ilo"], "ihi": c["ihi"], "disw": c["disw"],
            "ident": IDENT,
            "w1": np.ascontiguousarray(W1, dtype=np.float32),
            "b1": np.asarray(b1, dtype=np.float32).reshape(4, 125).T.copy(),
            "w23": np.asarray(w23, dtype=np.float32).reshape(4, 125).T.copy(),
        })
    return maps


def pass2_inmaps(pp, Zpad):
    maps = []
    for k in range(NCORES):
        c = pp["per_core"][k]
        maps.append({
            "xpad": Zpad,
            "ilo": c["ilo"], "ihi": c["ihi"], "disw": c["disw"],
        })
    return maps


def kernel(state, edge_attr, edge_index, W1, b1, W2, b2, W3, b3):
    state = np.asarray(state)
    edge_attr = np.asarray(edge_attr)
    edge_index = np.asarray(edge_index)
    pp = _preprocess(state, edge_attr, edge_index)
    w23 = np.asarray(W2, dtype=np.float32) @ np.asarray(W3, dtype=np.float32)
    c2 = float((np.asarray(b2, dtype=np.float32) @ np.asarray(W3, dtype=np.float32)
                + np.asarray(b3, dtype=np.float32))[0])

    X = np.concatenate([state.reshape(-1, edge_attr.shape[1]),
                        edge_attr], 0).astype(np.float32)
    U = X * pp["dis"][:, None]
    Upad = np.zeros((NROWS, D), dtype=np.float32)
    Upad[pp["rows"], :DF] = U

    nc1 = build_pass1(pp["KLO"], pp["KHI"])
    r1 = run_bass_kernel_spmd(nc1, pass1_inmaps(pp, Upad, W1, b1, w23),
                              core_ids=list(range(NCORES)))
    # z in label order: core k, within-core window wi, local p
    z_lbl = np.zeros(NPAD, dtype=np.float32)
    for k in range(NCORES):
        zc = r1.results[k]["z"][0][:WPC * P]            # [wi*128+p]
        j = (np.arange(WPC * P) // P) * NCORES + k      # sorted window id
        z_lbl[j * P + (np.arange(WPC * P) % P)] = zc
    member = pp["member"]
    real = member >= 0
    z_node = np.zeros(N, dtype=np.float32)
    z_node[member[real]] = z_lbl[real]
    zdis = z_node * pp["dis"]
    Zpad = np.zeros((NROWS, D), dtype=np.float32)
    Zpad[pp["rows"], 0] = zdis

    nc2 = build_pass2(pp["KLO"], pp["KHI"])
    r2 = run_bass_kernel_spmd(nc2, pass2_inmaps(pp, Zpad),
                              core_ids=list(range(NCORES)))
    y_lbl = np.zeros(NPAD, dtype=np.float32)
    for k in range(NCORES):
        yc = r2.results[k]["y"]                          # [128, 49]
        j = np.repeat(np.arange(WPC) * NCORES + k, P)
        y_lbl[j * P + np.tile(np.arange(P), WPC)] = yc.T.reshape(-1)
    y_node = np.zeros(N, dtype=np.float32)
    y_node[member[real]] = y_lbl[real]
    return (y_node + c2)[:, None].astype(np.float32)


# revision 15
# speedup vs baseline: 4.2392x; 4.2392x over previous
"""Trainium2 Bass kernel for nn_CriticGCN (2-layer GCN critic, 50000 nodes,
800000 edges, 8 NeuronCores).

Algebraic reformulation (exact):
  out1 = A @ X W1 + b1 with A = GCN-normalized adjacency (incl. self loops)
  z    = relu(out1) @ (W2 @ W3)          (layer-2 feature dim collapsed to 1)
  y    = A @ z + (b2 @ W3 + b3)
  A @ v per node d: dis[d] * (sum_{e: s->d} dis[s]*v[s] + dis[d]*v[d])

Layout strategy (no one-hot matmuls, no segment matmuls):
  - Nodes are relabeled by (lo_indeg, hi_indeg) lexicographic sort so each
    window of 128 dst nodes has near-uniform in-degree. Consecutive sorted
    windows are dealt round-robin to the 8 cores (load balance + identical
    shapes for SPMD).
  - Each dst node gets a fixed number of edge slots (window max degree incl.
    self loop); slot (p, t) of a window holds a gathered 256B row of the
    pre-scaled source table U = X*dis (or zdis = z*dis in pass 2). Padding
    slots point at an all-zeros table row.
  - The segment sum is then a plain free-dim tensor_reduce on the Vector
    engine: seg[p, f] = sum_t gathered[p, t, f]. Self loops are extra slots.
  - Gathers are grouped 7 windows per dma_gather call to amortize the SWDGE
    fixed cost; lo/hi table split works around the int16 index range.
  - Dense chain per 4 windows: transpose seg to [20, 512] via TensorE, then
    x1 = relu(W1^T g + b1) and z = w23^T x1 accumulated in PSUM.
"""
import numpy as np
import concourse.bacc as bacc
import concourse.mybir as mybir
import concourse.tile as tile
from concourse.bass_utils import run_bass_kernel_spmd

P = 128
NCORES = 8
WPC = 49                 # windows per core
NWIN = NCORES * WPC      # 392
NPAD = NWIN * P          # 50176
N = 50000
LOMAX = 32765            # node ids <= LOMAX live in the lo table region
ZLO = 32766              # lo region all-zeros pad row
HIBASE = 32767           # table row offset of the hi region
NHI = N - (LOMAX + 1)    # 17234 hi nodes (ids 32766..49999)
ZHI = NHI                # hi region pad idx (row HIBASE+NHI is zeros)
NROWS = HIBASE + NHI + 1 # 50002
D = 64                   # table row: 64 f32 = 256 B (dma_gather minimum)
DF = 20
GDW = 7                  # windows per dma_gather call
NGG = WPC // GDW         # 7 gather groups
GRP = 4                  # windows per dense-chain group
NGRP = (WPC + GRP - 1) // GRP
ZCOLS = NGRP * GRP * P

IDENT = np.eye(P, dtype=np.float32)


def _wrap16(a):
    return np.tile(a.astype(np.int16).reshape(-1, 16).T, (8, 1))


def _preprocess(state, edge_attr, edge_index):
    X = np.concatenate([state.reshape(-1, edge_attr.shape[1]),
                        edge_attr], 0).astype(np.float32)
    src = edge_index[0].astype(np.int64)
    dst = edge_index[1].astype(np.int64)

    deg = np.bincount(dst, minlength=N)
    dis = (1.0 / np.sqrt(deg + 1.0)).astype(np.float32)

    # edge lists split by src table region, grouped by dst (self loops are
    # handled by a per-window U[member] add after the reduce, not gathered)
    esrc = src
    edst = dst
    is_hi = esrc > LOMAX
    lo_src, lo_dst = esrc[~is_hi], edst[~is_hi]
    hi_src, hi_dst = esrc[is_hi], edst[is_hi]
    o = np.argsort(lo_dst, kind="stable")
    lo_src = lo_src[o]
    o = np.argsort(hi_dst, kind="stable")
    hi_src = hi_src[o] - (LOMAX + 1)
    lo_cnt = np.bincount(lo_dst, minlength=N)
    hi_cnt = np.bincount(hi_dst, minlength=N)
    lo_off = np.concatenate([[0], np.cumsum(lo_cnt)])
    hi_off = np.concatenate([[0], np.cumsum(hi_cnt)])

    # degree-homogeneous windows: sort by (lo_cnt, snake(hi_cnt)), pads first.
    # The snake (hi ascending for even lo, descending for odd) keeps hi_cnt
    # continuous across lo boundaries so the per-class max stays tight.
    hi_snake = np.where(lo_cnt % 2 == 0, hi_cnt, 1 << 20) - \
        np.where(lo_cnt % 2 == 0, 0, hi_cnt)
    order = np.lexsort((hi_snake, lo_cnt))
    member = np.full(NPAD, -1, dtype=np.int64)
    member[NPAD - N:] = order

    # per-window slot counts, then deal windows with similar (klo, khi) to
    # the same within-core index wi so the SPMD max over 8 cores stays tight
    mem2 = member.reshape(NWIN, P)
    klo_w = np.zeros(NWIN, dtype=np.int64)
    khi_w = np.zeros(NWIN, dtype=np.int64)
    for j in range(NWIN):
        m = mem2[j]
        real = m >= 0
        if real.any():
            klo_w[j] = lo_cnt[m[real]].max()
            khi_w[j] = hi_cnt[m[real]].max()
    khi_wsnake = np.where(klo_w % 2 == 0, khi_w, (1 << 20) - khi_w)
    worder = np.lexsort((khi_wsnake, klo_w))  # worder[wi*8 + k] = window of
    w2 = worder.reshape(WPC, NCORES)          # core k, within-core index wi
    KLO = np.maximum(klo_w[w2].max(1), 1)    # [49] per wi
    KHI = np.maximum(khi_w[w2].max(1), 1)

    def fill_rect(m, cnt_tab, off_tab, src_tab, k, padval):
        arr = np.full((k, P), padval, dtype=np.int64)
        if k == 0:
            return arr
        real = np.where(m >= 0)[0]
        mr = m[real]
        cnts = cnt_tab[mr]
        tot = int(cnts.sum())
        if tot == 0:
            return arr
        base = np.repeat(off_tab[mr], cnts)
        csum = np.concatenate([[0], np.cumsum(cnts)])[:-1]
        within = np.arange(tot) - np.repeat(csum, cnts)
        arr[within, np.repeat(real, cnts)] = src_tab[base + within]
        return arr

    # pre-scaled gather table U = X*dis in table-row layout
    U = X * dis[:, None]
    ids = np.arange(N)
    rows = np.where(ids <= LOMAX, ids, ids + 1)

    per_core = []
    disw_all = np.where(member >= 0, dis[np.clip(member, 0, None)],
                        1.0).astype(np.float32).reshape(NWIN, P)
    u_all = np.where((member >= 0)[:, None], U[np.clip(member, 0, None)],
                     0.0).astype(np.float32).reshape(NWIN, P, DF)
    for k in range(NCORES):
        ilo_parts, ihi_parts = [], []
        for wi in range(WPC):
            j = int(w2[wi, k])
            m = mem2[j]
            ilo_parts.append(
                fill_rect(m, lo_cnt, lo_off, lo_src, KLO[wi], ZLO).reshape(-1))
            ihi_parts.append(
                fill_rect(m, hi_cnt, hi_off, hi_src, KHI[wi], ZHI).reshape(-1))
        # one wrapped idx table per gather group, concatenated along cols
        ilo_cols, ihi_cols = [], []
        for g in range(NGG):
            lo_flat = np.concatenate(ilo_parts[g * GDW:(g + 1) * GDW])
            hi_flat = np.concatenate(ihi_parts[g * GDW:(g + 1) * GDW])
            ilo_cols.append(_wrap16(lo_flat))
            ihi_cols.append(_wrap16(hi_flat))
        wsel = w2[:, k]
        disw = disw_all[wsel].T.copy()                       # [128, 49]
        u = u_all[wsel].transpose(1, 0, 2).reshape(P, WPC * DF).copy()
        per_core.append(dict(ilo=np.concatenate(ilo_cols, axis=1),
                             ihi=np.concatenate(ihi_cols, axis=1),
                             disw=disw, u=u))

    return dict(per_core=per_core, member=member, dis=dis, rows=rows,
                KLO=KLO, KHI=KHI, w2=w2)


def _queue_plan(KLOG, KHIG):
    """Greedy bin-pack the per-group gathers onto 4 SWDGE queues (desc-gen
    for different queues runs on different Q7 core pairs, in parallel)."""
    items = [("lo", g, KLOG[g]) for g in range(NGG)] + \
            [("hi", g, KHIG[g]) for g in range(NGG)]
    items.sort(key=lambda t: -t[2])
    loads = [0] * 4
    qmap = {}
    for kind, g, wgt in items:
        q = loads.index(min(loads))
        qmap[(kind, g)] = q
        loads[q] += wgt
    return qmap


def _build_common(nc, KLO, KHI):
    """Declare the shared dram tensors + idx/disw constants; returns dict."""
    f32 = mybir.dt.float32
    nlo16 = int(sum(KLO)) * P // 16
    nhi16 = int(sum(KHI)) * P // 16
    t = {}
    t["xpad"] = nc.dram_tensor("xpad", [NROWS, D], f32, kind="ExternalInput")
    t["ilo"] = nc.dram_tensor("ilo", [128, nlo16], mybir.dt.int16,
                              kind="ExternalInput")
    t["ihi"] = nc.dram_tensor("ihi", [128, nhi16], mybir.dt.int16,
                              kind="ExternalInput")
    t["disw"] = nc.dram_tensor("disw", [P, WPC], f32, kind="ExternalInput")
    return t


def build_pass1(KLO, KHI, reps=1):
    KLO = [int(v) for v in KLO]
    KHI = [int(v) for v in KHI]
    KLOG = [sum(KLO[g * GDW:(g + 1) * GDW]) for g in range(NGG)]
    KHIG = [sum(KHI[g * GDW:(g + 1) * GDW]) for g in range(NGG)]
    KLOGM, KHIGM = max(KLOG), max(KHIG)
    lo_go = np.concatenate([[0], np.cumsum(KLOG)]) * P // 16
    hi_go = np.concatenate([[0], np.cumsum(KHIG)]) * P // 16
    qmap = _queue_plan(KLOG, KHIG)

    nc = bacc.Bacc("TRN2", target_bir_lowering=False, debug=False,
                   num_devices=NCORES, num_swdge_queues=4)
    f32 = mybir.dt.float32
    t = _build_common(nc, KLO, KHI)
    ident_d = nc.dram_tensor("ident", [P, P], f32, kind="ExternalInput")
    u_d = nc.dram_tensor("u", [P, WPC * DF], f32, kind="ExternalInput")
    w1_d = nc.dram_tensor("w1", [DF, 500], f32, kind="ExternalInput")
    b1_d = nc.dram_tensor("b1", [125, 4], f32, kind="ExternalInput")
    w23_d = nc.dram_tensor("w23", [125, 4], f32, kind="ExternalInput")
    z_d = nc.dram_tensor("z", [1, ZCOLS], f32, kind="ExternalOutput")

    Relu = mybir.ActivationFunctionType.Relu
    with tile.TileContext(nc) as tc:
        with tc.tile_pool(name="cst", bufs=1) as cst, \
             tc.tile_pool(name="g", bufs=2) as gpl, \
             tc.tile_pool(name="wk", bufs=3) as wk, \
             tc.tile_pool(name="ps", bufs=2, space="PSUM") as ps:
            ilo_t = cst.tile([128, t["ilo"].shape[1]], mybir.dt.int16)
            nc.sync.dma_start(out=ilo_t[:], in_=t["ilo"][:])
            ihi_t = cst.tile([128, t["ihi"].shape[1]], mybir.dt.int16)
            nc.sync.dma_start(out=ihi_t[:], in_=t["ihi"][:])
            disw_t = cst.tile([P, WPC], f32)
            nc.sync.dma_start(out=disw_t[:], in_=t["disw"][:])
            ident_t = cst.tile([P, P], f32)
            nc.sync.dma_start(out=ident_t[:], in_=ident_d[:])
            u_t = cst.tile([P, WPC * DF], f32)
            nc.sync.dma_start(out=u_t[:], in_=u_d[:])
            w1_t = cst.tile([DF, 500], f32)
            nc.sync.dma_start(out=w1_t[:], in_=w1_d[:])
            b1_t = cst.tile([125, 4], f32)
            nc.sync.dma_start(out=b1_t[:], in_=b1_d[:])
            w23_t = cst.tile([125, 4], f32)
            nc.sync.dma_start(out=w23_t[:], in_=w23_d[:])
            z_sb = cst.tile([1, ZCOLS], f32)

            for _rep in range(reps):
                gt_ps = None
                for g in range(NGG):
                    glo = gpl.tile([P, KLOGM, D], f32, tag="glo")
                    nc.gpsimd.dma_gather(
                        out_ap=glo[:, :KLOG[g], :],
                        in_ap=t["xpad"][:HIBASE, :],
                        idxs_ap=ilo_t[:, lo_go[g]:lo_go[g + 1]],
                        num_idxs=KLOG[g] * P, num_idxs_reg=KLOG[g] * P,
                        elem_size=D, single_packet=False,
                        queue_num=qmap[("lo", g)])
                    ghi = gpl.tile([P, KHIGM, D], f32, tag="ghi")
                    nc.gpsimd.dma_gather(
                        out_ap=ghi[:, :KHIG[g], :],
                        in_ap=t["xpad"][HIBASE:, :],
                        idxs_ap=ihi_t[:, hi_go[g]:hi_go[g + 1]],
                        num_idxs=KHIG[g] * P, num_idxs_reg=KHIG[g] * P,
                        elem_size=D, single_packet=False,
                        queue_num=qmap[("hi", g)])
                    olo, ohi = 0, 0
                    for wl in range(GDW):
                        w = g * GDW + wl
                        if w % GRP == 0:
                            gt_ps = ps.tile([DF, GRP * P], f32, space="PSUM",
                                            tag="gtps")
                        klo, khi = KLO[w], KHI[w]
                        seg = wk.tile([P, DF], f32, tag="seg")
                        rhi = wk.tile([P, DF], f32, tag="rhi")
                        nc.vector.tensor_reduce(
                            out=seg[:],
                            in_=glo[:, olo:olo + klo, :DF]
                                .rearrange("p k f -> p f k"),
                            axis=mybir.AxisListType.X, op=mybir.AluOpType.add)
                        nc.vector.tensor_reduce(
                            out=rhi[:],
                            in_=ghi[:, ohi:ohi + khi, :DF]
                                .rearrange("p k f -> p f k"),
                            axis=mybir.AxisListType.X, op=mybir.AluOpType.add)
                        olo += klo
                        ohi += khi
                        nc.vector.tensor_tensor(out=seg[:], in0=seg[:],
                                                in1=rhi[:],
                                                op=mybir.AluOpType.add)
                        nc.vector.tensor_tensor(out=seg[:], in0=seg[:],
                                                in1=u_t[:, w * DF:(w + 1) * DF],
                                                op=mybir.AluOpType.add)
                        nc.vector.tensor_tensor(
                            out=seg[:], in0=seg[:],
                            in1=disw_t[:, w:w + 1].to_broadcast([P, DF]),
                            op=mybir.AluOpType.mult)
                        wi = w % GRP
                        nc.tensor.transpose(out=gt_ps[:, wi * P:(wi + 1) * P],
                                            in_=seg[:], identity=ident_t[:])
                        if w % GRP == GRP - 1 or w == WPC - 1:
                            grp = w // GRP
                            gt_sb = wk.tile([DF, GRP * P], f32, tag="gtsb")
                            nc.vector.tensor_copy(out=gt_sb[:], in_=gt_ps[:])
                            zrow = ps.tile([1, GRP * P], f32, space="PSUM",
                                           tag="zrow")
                            for c in range(4):
                                o1 = ps.tile([125, GRP * P], f32,
                                             space="PSUM", tag="o1")
                                nc.tensor.matmul(
                                    out=o1[:],
                                    lhsT=w1_t[:, c * 125:(c + 1) * 125],
                                    rhs=gt_sb[:], start=True, stop=True)
                                x1 = wk.tile([125, GRP * P], f32, tag="x1")
                                nc.scalar.activation(x1[:], o1[:], Relu,
                                                     bias=b1_t[:, c:c + 1],
                                                     scale=1.0)
                                nc.tensor.matmul(out=zrow[:],
                                                 lhsT=w23_t[:, c:c + 1],
                                                 rhs=x1[:], start=(c == 0),
                                                 stop=(c == 3))
                            nc.vector.tensor_copy(
                                out=z_sb[:, grp * GRP * P:(grp + 1) * GRP * P],
                                in_=zrow[:])
            nc.sync.dma_start(out=z_d[:], in_=z_sb[:])
    nc.compile()
    return nc


def build_pass2(KLO, KHI, reps=1):
    KLO = [int(v) for v in KLO]
    KHI = [int(v) for v in KHI]
    KLOG = [sum(KLO[g * GDW:(g + 1) * GDW]) for g in range(NGG)]
    KHIG = [sum(KHI[g * GDW:(g + 1) * GDW]) for g in range(NGG)]
    KLOGM, KHIGM = max(KLOG), max(KHIG)
    lo_go = np.concatenate([[0], np.cumsum(KLOG)]) * P // 16
    hi_go = np.concatenate([[0], np.cumsum(KHIG)]) * P // 16
    qmap = _queue_plan(KLOG, KHIG)

    nc = bacc.Bacc("TRN2", target_bir_lowering=False, debug=False,
                   num_devices=NCORES, num_swdge_queues=4)
    f32 = mybir.dt.float32
    t = _build_common(nc, KLO, KHI)
    zw_d = nc.dram_tensor("zw", [P, WPC], f32, kind="ExternalInput")
    y_d = nc.dram_tensor("y", [P, WPC], f32, kind="ExternalOutput")

    with tile.TileContext(nc) as tc:
        with tc.tile_pool(name="cst", bufs=1) as cst, \
             tc.tile_pool(name="g", bufs=2) as gpl, \
             tc.tile_pool(name="wk", bufs=3) as wk:
            ilo_t = cst.tile([128, t["ilo"].shape[1]], mybir.dt.int16)
            nc.sync.dma_start(out=ilo_t[:], in_=t["ilo"][:])
            ihi_t = cst.tile([128, t["ihi"].shape[1]], mybir.dt.int16)
            nc.sync.dma_start(out=ihi_t[:], in_=t["ihi"][:])
            disw_t = cst.tile([P, WPC], f32)
            nc.sync.dma_start(out=disw_t[:], in_=t["disw"][:])
            zw_t = cst.tile([P, WPC], f32)
            nc.sync.dma_start(out=zw_t[:], in_=zw_d[:])
            y_sb = cst.tile([P, WPC], f32)

            for _rep in range(reps):
                for g in range(NGG):
                    glo = gpl.tile([P, KLOGM, D], f32, tag="glo")
                    nc.gpsimd.dma_gather(
                        out_ap=glo[:, :KLOG[g], :],
                        in_ap=t["xpad"][:HIBASE, :],
                        idxs_ap=ilo_t[:, lo_go[g]:lo_go[g + 1]],
                        num_idxs=KLOG[g] * P, num_idxs_reg=KLOG[g] * P,
                        elem_size=D, single_packet=False,
                        queue_num=qmap[("lo", g)])
                    ghi = gpl.tile([P, KHIGM, D], f32, tag="ghi")
                    nc.gpsimd.dma_gather(
                        out_ap=ghi[:, :KHIG[g], :],
                        in_ap=t["xpad"][HIBASE:, :],
                        idxs_ap=ihi_t[:, hi_go[g]:hi_go[g + 1]],
                        num_idxs=KHIG[g] * P, num_idxs_reg=KHIG[g] * P,
                        elem_size=D, single_packet=False,
                        queue_num=qmap[("hi", g)])
                    olo, ohi = 0, 0
                    for wl in range(GDW):
                        w = g * GDW + wl
                        klo, khi = KLO[w], KHI[w]
                        seg = wk.tile([P, 1], f32, tag="seg")
                        rhi = wk.tile([P, 1], f32, tag="rhi")
                        nc.vector.tensor_reduce(
                            out=seg[:],
                            in_=glo[:, olo:olo + klo, 0:1]
                                .rearrange("p k f -> p f k"),
                            axis=mybir.AxisListType.X, op=mybir.AluOpType.add)
                        nc.vector.tensor_reduce(
                            out=rhi[:],
                            in_=ghi[:, ohi:ohi + khi, 0:1]
                                .rearrange("p k f -> p f k"),
                            axis=mybir.AxisListType.X, op=mybir.AluOpType.add)
                        olo += klo
                        ohi += khi
                        nc.vector.tensor_tensor(out=seg[:], in0=seg[:],
                                                in1=rhi[:],
                                                op=mybir.AluOpType.add)
                        nc.vector.tensor_tensor(out=seg[:], in0=seg[:],
                                                in1=zw_t[:, w:w + 1],
                                                op=mybir.AluOpType.add)
                        nc.vector.tensor_tensor(
                            out=y_sb[:, w:w + 1], in0=seg[:],
                            in1=disw_t[:, w:w + 1], op=mybir.AluOpType.mult)
            nc.sync.dma_start(out=y_d[:], in_=y_sb[:])
    nc.compile()
    return nc


def pass1_inmaps(pp, Upad, W1, b1, w23):
    maps = []
    for k in range(NCORES):
        c = pp["per_core"][k]
        maps.append({
            "xpad": Upad,
            "ilo": c["ilo"], "ihi": c["ihi"], "disw": c["disw"],
            "ident": IDENT, "u": c["u"],
            "w1": np.ascontiguousarray(W1, dtype=np.float32),
            "b1": np.asarray(b1, dtype=np.float32).reshape(4, 125).T.copy(),
            "w23": np.asarray(w23, dtype=np.float32).reshape(4, 125).T.copy(),
        })
    return maps


def pass2_inmaps(pp, Zpad, zw_per_core=None):
    maps = []
    for k in range(NCORES):
        c = pp["per_core"][k]
        zw = (zw_per_core[k] if zw_per_core is not None
              else np.zeros((P, WPC), dtype=np.float32))
        maps.append({
            "xpad": Zpad,
            "ilo": c["ilo"], "ihi": c["ihi"], "disw": c["disw"], "zw": zw,
        })
    return maps


def kernel(state, edge_attr, edge_index, W1, b1, W2, b2, W3, b3):
    state = np.asarray(state)
    edge_attr = np.asarray(edge_attr)
    edge_index = np.asarray(edge_index)
    pp = _preprocess(state, edge_attr, edge_index)
    w23 = np.asarray(W2, dtype=np.float32) @ np.asarray(W3, dtype=np.float32)
    c2 = float((np.asarray(b2, dtype=np.float32) @ np.asarray(W3, dtype=np.float32)
                + np.asarray(b3, dtype=np.float32))[0])

    X = np.concatenate([state.reshape(-1, edge_attr.shape[1]),
                        edge_attr], 0).astype(np.float32)
    U = X * pp["dis"][:, None]
    Upad = np.zeros((NROWS, D), dtype=np.float32)
    Upad[pp["rows"], :DF] = U

    nc1 = build_pass1(pp["KLO"], pp["KHI"])
    r1 = run_bass_kernel_spmd(nc1, pass1_inmaps(pp, Upad, W1, b1, w23),
                              core_ids=list(range(NCORES)))
    # z in label order: core k, within-core window wi, local p
    w2 = pp["w2"]
    z_lbl = np.zeros(NPAD, dtype=np.float32)
    for k in range(NCORES):
        zc = r1.results[k]["z"][0][:WPC * P]            # [wi*128+p]
        j = w2[np.arange(WPC * P) // P, k]              # sorted window id
        z_lbl[j * P + (np.arange(WPC * P) % P)] = zc
    member = pp["member"]
    real = member >= 0
    z_node = np.zeros(N, dtype=np.float32)
    z_node[member[real]] = z_lbl[real]
    zdis = z_node * pp["dis"]
    Zpad = np.zeros((NROWS, D), dtype=np.float32)
    Zpad[pp["rows"], 0] = zdis

    # per-core self-loop term zdis[member] in [p, wi] layout
    zdis_lbl = np.where(real, zdis[np.clip(member, 0, None)], 0.0)
    zw_per_core = []
    for k in range(NCORES):
        wsel = w2[:, k]
        zw_per_core.append(
            zdis_lbl.reshape(NWIN, P)[wsel].T.astype(np.float32).copy())

    nc2 = build_pass2(pp["KLO"], pp["KHI"])
    r2 = run_bass_kernel_spmd(nc2, pass2_inmaps(pp, Zpad, zw_per_core),
                              core_ids=list(range(NCORES)))
    y_lbl = np.zeros(NPAD, dtype=np.float32)
    for k in range(NCORES):
        yc = r2.results[k]["y"]                          # [128, 49]
        j = np.repeat(w2[:, k], P)
        y_lbl[j * P + np.tile(np.arange(P), WPC)] = yc.T.reshape(-1)
    y_node = np.zeros(N, dtype=np.float32)
    y_node[member[real]] = y_lbl[real]
    return (y_node + c2)[:, None].astype(np.float32)
